# revision 1
# baseline (speedup 1.0000x reference)
"""BlockGCN Trainium2 kernel.

Math (reference): per (k,h): graph-mix over nodes with BnA[k,h] (23x23),
grouped 1x1 conv (16->16 per head), sum over k, BN(eval), relu(y + x).

Restructured for TRN2 (per batch n):
  natural layout: SBUF tile (C=128 partitions, T*V=5888 free), contiguous DMA.
  chunk = 115 free elems = 5 t x 23 v (t-major flat index t*23+v).
  stage A (conv + transpose, one matmul per chunk):
      YT[tv, k*128+o] = sum_c x[c, tv] * Wcat[c, k*128+o]
      via matmul(lhsT=x_chunk (128c,115tv), rhs=Wcat (128c,384)).
      Wcat = block-diag conv weights, pre-scaled by BN inv (folded).
  stage B (graph-mix, k-summed in PSUM, per head h and chunk-group):
      out2[(i,w),(j,c_h)] += sum_(i,v) BnAbig[k,h][(i,v),(i,w)] * YT_k[(i,v),(j,c_h)]
      via matmul(lhsT=BnAbig (115,115), rhs=YT composite slice (115, nj*16)).
      BnAbig = kron(I5, BnA[k,h]); block-diagonal => the 5-t groups and the
      garbage tail columns never mix.
  back-T + residual (per chunk, accumulated in PSUM):
      out3 = out2_chunk^T  (matmul lhsT=out2_chunk (115,128), rhs=I115)
           + x_chunk       (matmul lhsT=I128, rhs=x_chunk)
  epilogue: out = Relu(out3 + Cvec)  on ScalarE, PSUM -> SBUF fp32, then
      one contiguous 3MB store per batch.
  BN/bias folding: inv = gamma/sqrt(var+eps) folded into Wcat;
      Cvec = inv*sum_k(conv_b) + beta - mean*inv  (per-partition ACT bias).

Sharding: data-parallel over batch N=32 across 8 cores (4 each).
"""

import numpy as np
import ml_dtypes

V = 23
K = 3
H = 8
C = 128
T = 256
N = 32
CG = C // H            # 16
TV = T * V             # 5888
CH = 115               # chunk: 5 t x 23 v
NCH = 52               # ceil(5888/115); last chunk has 92 garbage cols
TVP = NCH * CH         # 5980 padded free size
CORES = 8
NPC = N // CORES       # 4 batches per core
BN_EPS = 1e-5

_CACHE = {}


def _build_nc(repeat=1, acta=6, actn=12, epi_dve=5, epin=13, psa=4, psb=2, psc=2, bdve=10):
    # acta/actn: fraction of stage-A copies on ACT (acta of every actn)
    # epi_dve/epin: fraction of epilogue macros on DVE
    import concourse.bass as bass
    import concourse.tile as tile
    from concourse import bacc, mybir

    f32 = mybir.dt.float32
    bf16 = mybir.dt.bfloat16

    nc = bacc.Bacc(None, target_bir_lowering=False)

    x_d = nc.declare_dram_parameter("x", [NPC, C, TV], bf16, isOutput=False)
    wcat_d = nc.declare_dram_parameter("wcat", [C, K * C], bf16, isOutput=False)
    bna_d = nc.declare_dram_parameter("bna", [CH, K * H, CH], bf16, isOutput=False)
    i115_d = nc.declare_dram_parameter("i115", [CH, CH], bf16, isOutput=False)
    i128_d = nc.declare_dram_parameter("i128", [C, C], bf16, isOutput=False)
    cvec_d = nc.declare_dram_parameter("cvec", [C, 1], f32, isOutput=False)
    out_d = nc.declare_dram_parameter("out", [NPC, C, TV], f32, isOutput=True)

    # stage-B chunk groups (free dim = nj*16 <= 512)
    groups = [(0, 32), (32, 20)]
    # back-T macro groups: 4 chunks -> one 460-col psum bank
    MAC = 4
    NMAC = NCH // MAC  # 13

    with tile.TileContext(nc) as tc:
        with (
            tc.tile_pool(name="consts", bufs=1) as consts,
            tc.tile_pool(name="xp", bufs=2) as xp,
            tc.tile_pool(name="ytp", bufs=2) as ytp,
            tc.tile_pool(name="o2p", bufs=2) as o2p,
            tc.tile_pool(name="outp", bufs=2) as outp,
            tc.tile_pool(name="psA", bufs=psa, space="PSUM") as psA,
            tc.tile_pool(name="psB", bufs=psb, space="PSUM") as psB,
            tc.tile_pool(name="psC", bufs=psc, space="PSUM") as psC,
        ):
            # consts on the ACT HWDGE ring so the first x load (SP ring)
            # is not queued behind them
            wcat_sb = consts.tile([C, K * C], bf16)
            nc.scalar.dma_start(wcat_sb[:], wcat_d[:])
            bna_sb = consts.tile([CH, K * H, CH], bf16)
            nc.scalar.dma_start(bna_sb[:], bna_d[:])
            i115_sb = consts.tile([CH, CH], bf16)
            nc.scalar.dma_start(i115_sb[:], i115_d[:])
            i128_sb = consts.tile([C, C], bf16)
            nc.scalar.dma_start(i128_sb[:], i128_d[:])
            cvec_sb = consts.tile([C, 1], f32)
            nc.scalar.dma_start(cvec_sb[:], cvec_d[:])

            for n in [i for _ in range(repeat) for i in range(NPC)]:
                io = outp.tile([C, TVP], mybir.dt.float32)
                xbf = xp.tile([C, TVP], bf16)

                def xchunk(j):
                    return xbf[:, j * CH:(j + 1) * CH]

                # zero the pad tail so 0*garbage can't inject NaN into the
                # chunk-51 contraction
                nc.gpsimd.memset(xbf[:, TV:], 0.0)
                # split load so stage A can start on the first piece
                for (a, b) in ((0, 2 * CH), (2 * CH, 10 * CH), (10 * CH, 26 * CH),
                               (26 * CH, TV)):
                    nc.sync.dma_start(xbf[:, a:b], x_d[n][:, a:b])

                # ---- stage A ----
                ytall = ytp.tile([CH, NCH, K * C], bf16)
                for j in range(NCH):
                    pyt = psA.tile([CH, K * C], mybir.dt.float32)
                    nc.tensor.matmul(
                        pyt[:],
                        xchunk(j),
                        wcat_sb[:],
                        start=True, stop=True,
                    )
                    # interleaved weighted split between ACT and DVE
                    if (j * acta) % actn < acta:
                        nc.scalar.copy(ytall[:, j, :], pyt[:])
                    else:
                        nc.vector.tensor_copy(ytall[:, j, :], pyt[:])

                # ---- stage B + C interleaved at group granularity ----
                # group 0 (chunks 0..31) completes across all h first, then
                # its back-T/epilogue macros are emitted before group 1 so
                # the scheduler prioritizes draining them
                o2 = o2p.tile([CH, NCH, C], bf16)
                cnt = 0

                def stage_b_group(j0, nj):
                    nonlocal cnt
                    for h in range(H):
                        po2 = psB.tile([CH, 512], mybir.dt.float32)
                        for k in range(K):
                            nc.tensor.matmul(
                                po2[:, :nj * CG],
                                bna_sb[:, k * H + h, :],
                                ytall[:, j0:j0 + nj, k * C + h * CG: k * C + (h + 1) * CG],
                                start=(k == 0), stop=(k == K - 1),
                            )
                        if (cnt * bdve) % 16 < bdve:
                            nc.vector.tensor_copy(
                                o2[:, j0:j0 + nj, h * CG:(h + 1) * CG], po2[:, :nj * CG])
                        else:
                            nc.scalar.copy(
                                o2[:, j0:j0 + nj, h * CG:(h + 1) * CG], po2[:, :nj * CG])
                        cnt += 1

                def stage_c(qa, qb):
                    for q in range(qa, qb):
                        po3 = psC.tile([C, MAC * CH], mybir.dt.float32)
                        for jj in range(MAC):
                            j = q * MAC + jj
                            sl = po3[:, jj * CH:(jj + 1) * CH]
                            nc.tensor.matmul(
                                sl, o2[:, j, :], i115_sb[:],
                                start=True, stop=False)
                            nc.tensor.matmul(
                                sl, i128_sb[:], xchunk(j),
                                start=False, stop=True)
                        dst = io[:, q * MAC * CH:(q + 1) * MAC * CH]
                        if (q * epi_dve) % epin < epi_dve:
                            nc.vector.tensor_scalar(
                                dst, po3[:], cvec_sb[:], 0.0,
                                mybir.AluOpType.add, mybir.AluOpType.max)
                        else:
                            nc.scalar.activation(
                                dst, po3[:],
                                mybir.ActivationFunctionType.Relu,
                                bias=cvec_sb[:],
                            )

                stage_b_group(*groups[0])
                stage_b_group(*groups[1])
                stage_c(0, NMAC)
                # split stores so they start as epilogue macros complete
                for (qa, qb) in ((0, 4), (4, 8), (8, 11), (11, NMAC)):
                    ca, cb = qa * MAC * CH, min(qb * MAC * CH, TV)
                    nc.sync.dma_start(out_d[n][:, ca:cb], io[:, ca:cb])

    nc.compile()
    return nc


def _host_constants(emb_table, A, conv_w, conv_b, bn_gamma, bn_beta,
                    bn_mean, bn_var, hop):
    B = emb_table[:, :, hop]                                    # (k,h,v,v)
    l2 = lambda w: np.sqrt((w * w).sum(-2, keepdims=True)) + 1e-4
    BnA = (B / l2(B) + A / l2(A)).astype(np.float32)            # (k,h,v,v)
    inv = (bn_gamma / np.sqrt(bn_var + BN_EPS)).astype(np.float32)
    Bsum = conv_b.reshape(K, C).sum(0)
    cvec = (inv * Bsum + (bn_beta - bn_mean * inv)).astype(np.float32)

    wcat = np.zeros((C, K * C), np.float32)
    for k in range(K):
        for h in range(H):
            blk = conv_w[k * C + h * CG:k * C + (h + 1) * CG, :]   # (o16, c16)
            wcat[h * CG:(h + 1) * CG, k * C + h * CG:k * C + (h + 1) * CG] = (
                blk * inv[h * CG:(h + 1) * CG][:, None]).T

    # bna host layout: [p=115, k*H+h, f=115], BnAbig = kron(I5, BnA[k,h])
    bna = np.zeros((CH, K * H, CH), np.float32)
    for k in range(K):
        for h in range(H):
            big = np.kron(np.eye(CH // V, dtype=np.float32), BnA[k, h])
            bna[:, k * H + h, :] = big

    bf = ml_dtypes.bfloat16
    return {
        "wcat": wcat.astype(bf),
        "bna": bna.astype(bf),
        "i115": np.eye(CH, dtype=np.float32).astype(bf),
        "i128": np.eye(C, dtype=np.float32).astype(bf),
        "cvec": cvec.reshape(C, 1).astype(np.float32),
    }


def kernel(x, emb_table, A, conv_w, conv_b, bn_gamma, bn_beta, bn_mean,
           bn_var, hop):
    from concourse.bass_utils import run_bass_kernel_spmd

    x = np.ascontiguousarray(
        np.asarray(x, dtype=np.float32).astype(ml_dtypes.bfloat16))
    consts = _host_constants(
        np.asarray(emb_table, np.float32), np.asarray(A, np.float32),
        np.asarray(conv_w, np.float32), np.asarray(conv_b, np.float32),
        np.asarray(bn_gamma, np.float32), np.asarray(bn_beta, np.float32),
        np.asarray(bn_mean, np.float32), np.asarray(bn_var, np.float32),
        np.asarray(hop))

    if "nc" not in _CACHE:
        _CACHE["nc"] = _build_nc()
    nc = _CACHE["nc"]

    xs = x.reshape(N, C, TV)
    in_maps = [
        {"x": np.ascontiguousarray(xs[i * NPC:(i + 1) * NPC]), **consts}
        for i in range(CORES)
    ]
    res = run_bass_kernel_spmd(nc, in_maps, list(range(CORES)))
    out = np.concatenate([res.results[i]["out"] for i in range(CORES)], axis=0)
    return out.reshape(N, C, T, V).astype(np.float32)

